# revision 1
# baseline (speedup 1.0000x reference)
"""Trainium2 Bass kernel for nn_MeshConv (ChebConv K=2, two layers) on 8 cores.

Math (reference):
    deg  = bincount(src)                          # out-degree over src column
    dinv = where(deg>0, rsqrt(max(deg,1)), 0)
    z    = segment_sum(-dinv[src]*dinv[dst] * x[src], dst)
         = -dinv[dst] * segment_sum((dinv*x)[src], dst)
    layer(x) = x @ W0 + z-term @ W1 + b     (layer1 wrapped in relu)

Device strategy (per core, dst-sharded; two dispatches with a host hop for
the full layer-1 activation):
  * gather tables T[v] = dinv[v] * (x @ W1)  built on device as bf16 rows of
    256B (real cols 0:64 / 0:32), split in 4 chunks of <=25216 rows so the
    MoE dma_gather ucode op (int16 indices) can address them.
  * edges sorted by (dst-block, src-chunk); each 128-dst block accumulates
    sum_{e} T[src_e] one-hot-wise:  z^T = sum_t G_t^T @ S_t  on the PE,
    where G_t is a 128-edge gathered tile and S_t[e, m] = (slot_e == m).
  * epilogue: out^T = dense^T - dinv[dst] * z^T  (+relu on layer 1).
"""
import os
import numpy as np
import ml_dtypes

import concourse.bacc as bacc
import concourse.tile as tile
import concourse.mybir as mybir
import concourse.bass as bass
from concourse import library_config
from concourse.bass_utils import run_bass_kernel_spmd

P = 128
RW = 128                     # table row width in bf16 = 256B

# exported for test.py: exec times of the two dispatches when tracing
LAST_EXEC_NS = []


class Cfg:
    def __init__(self, n_nodes, n_edges, n_cores, in_dim, h1, h2):
        self.N, self.E, self.C = n_nodes, n_edges, n_cores
        self.IN, self.H1, self.H2 = in_dim, h1, h2
        self.OWN = n_nodes // n_cores                 # owned dst nodes / core
        self.NB = -(-self.OWN // P)                   # dst blocks / core
        self.NODES_PAD = self.NB * P
        # src chunking for int16 gather indices
        self.NCH = -(-n_nodes // 25088) if n_nodes > 25088 else 1
        self.CHUNK_REAL = -(-n_nodes // self.NCH)
        self.CHUNK_REAL = -(-self.CHUNK_REAL // P) * P   # 128-aligned
        self.CHUNK_ROWS = self.CHUNK_REAL + P            # + zero pad tile
        assert self.CHUNK_ROWS <= 32767
        self.PAD_ROW = self.CHUNK_REAL                   # chunk-local zero row
        self.NT = (self.NCH * self.CHUNK_REAL) // P      # node tiles (global)
        self.NT_CH = self.CHUNK_REAL // P                # node tiles / chunk
        self.NPADCOL = self.NCH * self.CHUNK_REAL        # padded node count
        # gather call grouping
        self.GRP = 6
        self.GROUPS = []
        b = 0
        while b < self.NB:
            n = min(self.GRP, self.NB - b)
            self.GROUPS.append((b, n))
            b += n

    def set_tbq(self, t_bq):
        self.T_BQ = t_bq
        self.CAP = t_bq * P
        self.TOTAL = self.NB * self.NCH * self.CAP       # idx slots / layer
        self.TTOT = self.TOTAL // P


def _build_layer(cfg, kf, m_out, relu, out_f32):
    """One dispatch: table build + dense + gather/segment-matmul + epilogue.

    kf: input feature dim (192 layer1 / 64 layer2); m_out: 64 / 32.
    """
    c = cfg
    nc = bacc.Bacc("TRN2", target_bir_lowering=False, debug=False)
    dt = mybir.dt

    vt = nc.dram_tensor("vt", [kf, c.NPADCOL], dt.bfloat16, kind="ExternalInput")
    w_t = nc.dram_tensor("w_t", [kf, m_out], dt.bfloat16, kind="ExternalInput")
    w_d = nc.dram_tensor("w_d", [kf, m_out], dt.bfloat16, kind="ExternalInput")
    bias = nc.dram_tensor("bias", [m_out, 1], dt.float32, kind="ExternalInput")
    deg_t = nc.dram_tensor("deg_t", [P, c.NT], dt.float32, kind="ExternalInput")
    degd = nc.dram_tensor("degd", [m_out, c.NODES_PAD], dt.float32,
                          kind="ExternalInput")
    idx = nc.dram_tensor("idx", [P, c.TOTAL // 16], dt.int16, kind="ExternalInput")
    slot = nc.dram_tensor("slot", [P, c.TTOT], dt.bfloat16, kind="ExternalInput")
    iota = nc.dram_tensor("iota", [P, P], dt.bfloat16, kind="ExternalInput")
    odt = dt.float32 if out_f32 else dt.bfloat16
    out = nc.dram_tensor("out", [m_out, c.NODES_PAD], odt, kind="ExternalOutput")

    tables = [
        nc.dram_tensor(f"table{q}", [c.CHUNK_ROWS, RW], dt.bfloat16)
        for q in range(c.NCH)
    ]

    kchunks = []           # (row0, rows) feature chunks for contraction
    r = 0
    while r < kf:
        n = min(P, kf - r)
        kchunks.append((r, n))
        r += n

    own_lo = None  # own column offset passed via python closure per-core? No:
    # own columns are a per-core slice of vt; but the program must be SPMD-
    # identical. We instead require the host to place the own-node columns at
    # a fixed location: host ships a separate dense input.
    vox = nc.dram_tensor("vox", [kf, c.NODES_PAD], dt.bfloat16,
                         kind="ExternalInput")

    with tile.TileContext(nc) as tc:
        with tc.tile_pool(name="const", bufs=1) as cpool:
            nc.gpsimd.load_library(library_config.mlp)

            iota_t = cpool.tile([P, 1, P], dt.bfloat16)
            nc.sync.dma_start(iota_t[:, 0, :], iota[:, :])
            bias_t = cpool.tile([m_out, 1], dt.float32)
            nc.sync.dma_start(bias_t[:], bias[:, :])
            wt_t = [cpool.tile([n, m_out], dt.bfloat16, tag=f"wt{i}", name=f"wt{i}")
                    for i, (r0, n) in enumerate(kchunks)]
            wd_t = [cpool.tile([n, m_out], dt.bfloat16, tag=f"wd{i}", name=f"wd{i}")
                    for i, (r0, n) in enumerate(kchunks)]
            for i, (r0, n) in enumerate(kchunks):
                nc.sync.dma_start(wt_t[i][:], w_t[r0:r0 + n, :])
                nc.sync.dma_start(wd_t[i][:], w_d[r0:r0 + n, :])

            # dinv for table rows: rsqrt(max(deg,1)) in node-tile layout
            dinv_t = cpool.tile([P, c.NT], dt.float32)
            degt_s = cpool.tile([P, c.NT], dt.float32)
            nc.sync.dma_start(degt_s[:], deg_t[:, :])
            nc.vector.tensor_scalar(out=degt_s[:], in0=degt_s[:], scalar1=1.0,
                                    scalar2=None, op0=mybir.AluOpType.max)
            nc.vector.reciprocal(out=dinv_t[:], in_=degt_s[:])
            nc.scalar.activation(out=dinv_t[:], in_=dinv_t[:],
                                 func=mybir.ActivationFunctionType.Sqrt)

            # dinv replicated over feature rows for the epilogue, with the
            # deg>0 mask:  dr = min(deg,1) * rsqrt(max(deg,1)); chunked to
            # keep f32 temporaries small
            dinv_rep = cpool.tile([m_out, c.NODES_PAD], dt.bfloat16)
            dense_t = cpool.tile([m_out, c.NODES_PAD], dt.bfloat16)
            DJ = 512
            with tc.tile_pool(name="drp", bufs=2) as dpool:
                for j in range(0, c.NODES_PAD, DJ):
                    dj = min(DJ, c.NODES_PAD - j)
                    dr_f = dpool.tile([m_out, dj], dt.float32, tag="drf")
                    dr_m = dpool.tile([m_out, dj], dt.float32, tag="drm")
                    nc.sync.dma_start(dr_f[:], degd[:, j:j + dj])
                    nc.vector.tensor_scalar(
                        out=dr_m[:], in0=dr_f[:], scalar1=1.0,
                        scalar2=None, op0=mybir.AluOpType.min)
                    nc.vector.tensor_scalar(
                        out=dr_f[:], in0=dr_f[:], scalar1=1.0,
                        scalar2=None, op0=mybir.AluOpType.max)
                    nc.vector.reciprocal(out=dr_f[:], in_=dr_f[:])
                    nc.scalar.activation(
                        out=dr_f[:], in_=dr_f[:],
                        func=mybir.ActivationFunctionType.Sqrt)
                    nc.vector.tensor_tensor(
                        out=dinv_rep[:, j:j + dj], in0=dr_f[:], in1=dr_m[:],
                        op=mybir.AluOpType.mult)

            # ---- table build (chunk-major so gathers can start early) ----
            with tc.tile_pool(name="bld", bufs=3) as bpool, \
                 tc.tile_pool(name="bpsum", bufs=4, space="PSUM") as bpsum:
                TB = 4
                assert c.NT_CH % TB == 0
                for t0 in range(0, c.NT, TB):
                    q = t0 // c.NT_CH
                    vtiles = [bpool.tile([n, TB * P], dt.bfloat16,
                                         tag=f"v{i}", name=f"v{i}")
                              for i, (r0, n) in enumerate(kchunks)]
                    for i, (r0, n) in enumerate(kchunks):
                        nc.sync.dma_start(
                            vtiles[i][:], vt[r0:r0 + n, t0 * P:(t0 + TB) * P])
                    for k in range(TB):
                        t = t0 + k
                        tq = t % c.NT_CH
                        ps = bpsum.tile([P, m_out], dt.float32, space="PSUM")
                        for i, (r0, n) in enumerate(kchunks):
                            nc.tensor.matmul(
                                out=ps[:], lhsT=vtiles[i][:, k * P:(k + 1) * P],
                                rhs=wt_t[i][:], start=(i == 0),
                                stop=(i == len(kchunks) - 1))
                        stg = bpool.tile([P, RW], dt.bfloat16)
                        nc.vector.memset(stg[:, m_out:RW], 0)
                        nc.vector.tensor_scalar(
                            out=stg[:, 0:m_out], in0=ps[:],
                            scalar1=dinv_t[:, t:t + 1], scalar2=None,
                            op0=mybir.AluOpType.mult)
                        nc.sync.dma_start(
                            tables[q][tq * P:(tq + 1) * P, :], stg[:])
                for q in range(c.NCH):
                    zstg = bpool.tile([P, RW], dt.bfloat16, tag="z")
                    nc.vector.memset(zstg[:], 0)
                    nc.sync.dma_start(
                        tables[q][c.CHUNK_REAL:c.CHUNK_ROWS, :], zstg[:])

                # ---- dense term: dense^T = W_d^T x^T + b ----
                for j in range(0, c.NODES_PAD, DJ):
                    dj = min(DJ, c.NODES_PAD - j)
                    ps = bpsum.tile([m_out, dj], dt.float32, space="PSUM",
                                    tag="dps")
                    for i, (r0, n) in enumerate(kchunks):
                        vtile = bpool.tile([n, dj], dt.bfloat16, tag=f"dv{i}")
                        nc.sync.dma_start(vtile[:], vox[r0:r0 + n, j:j + dj])
                        nc.tensor.matmul(out=ps[:], lhsT=wd_t[i][:],
                                         rhs=vtile[:], start=(i == 0),
                                         stop=(i == len(kchunks) - 1))
                    nc.vector.tensor_scalar(
                        out=dense_t[:, j:j + dj], in0=ps[:],
                        scalar1=bias_t[:, 0:1], scalar2=None,
                        op0=mybir.AluOpType.add)

            # ---- gather + segment matmul + epilogue ----
            with tc.tile_pool(name="gat", bufs=2) as gpool, \
                 tc.tile_pool(name="eppool", bufs=4) as epool, \
                 tc.tile_pool(name="gpsum", bufs=6, space="PSUM") as gpsum:
                goff = 0       # idx entries consumed so far
                for (b0, nblk) in c.GROUPS:
                    nidx = nblk * c.CAP
                    tg = nidx // P
                    zts = [gpsum.tile([m_out, P], dt.float32, space="PSUM",
                                      tag="zt", name="zt") for _ in range(nblk)]
                    for q in range(c.NCH):
                        i0 = goff + q * nidx
                        idx_t = gpool.tile([P, nidx // 16], dt.int16, tag="ix")
                        nc.sync.dma_start(
                            idx_t[:], idx[:, i0 // 16:(i0 + nidx) // 16])
                        slot_t = gpool.tile([P, tg], dt.bfloat16, tag="sl")
                        nc.sync.dma_start(
                            slot_t[:], slot[:, i0 // P:(i0 + nidx) // P])
                        g_t = gpool.tile([P, tg, RW], dt.bfloat16, tag="g")
                        SUB = 1024
                        for sb in range(0, nidx, SUB):
                            sn = min(SUB, nidx - sb)
                            nc.gpsimd.dma_gather(
                                g_t[:, sb // P:(sb + sn) // P, :],
                                tables[q][:, :],
                                idx_t[:, sb // 16:(sb + sn) // 16],
                                sn, sn, RW)
                        s_oh = gpool.tile([P, tg, P], dt.bfloat16, tag="s")
                        nc.vector.tensor_tensor(
                            out=s_oh[:],
                            in0=slot_t[:].to_broadcast([P, tg, P]),
                            in1=iota_t[:].to_broadcast([P, tg, P]),
                            op=mybir.AluOpType.is_equal)
                        for br in range(nblk):
                            for tr in range(c.T_BQ):
                                tt = br * c.T_BQ + tr
                                nc.tensor.matmul(
                                    out=zts[br][:],
                                    lhsT=g_t[:, tt, 0:m_out],
                                    rhs=s_oh[:, tt, :],
                                    start=(q == 0 and tr == 0),
                                    stop=(q == c.NCH - 1 and tr == c.T_BQ - 1))
                    for br in range(nblk):
                        blk = b0 + br
                        js = slice(blk * P, (blk + 1) * P)
                        tmp = epool.tile([m_out, P], dt.float32, tag="tmp")
                        nc.vector.tensor_tensor(out=tmp[:], in0=zts[br][:],
                                                in1=dinv_rep[:, js],
                                                op=mybir.AluOpType.mult)
                        ob = epool.tile([m_out, P], odt, tag="ob")
                        if relu:
                            nc.vector.tensor_tensor(
                                out=tmp[:], in0=dense_t[:, js], in1=tmp[:],
                                op=mybir.AluOpType.subtract)
                            nc.vector.tensor_scalar(
                                out=ob[:], in0=tmp[:], scalar1=0.0,
                                scalar2=None, op0=mybir.AluOpType.max)
                        else:
                            nc.vector.tensor_tensor(
                                out=ob[:], in0=dense_t[:, js],
                                in1=tmp[:], op=mybir.AluOpType.subtract)
                        nc.sync.dma_start(out[:, js], ob[:])
                    goff += c.NCH * nidx
    nc.compile()
    return nc


def _schedule(cfg, es, ed):
    """Per-core edge schedule. es: src node ids (global), ed: local dst ids.
    Returns int16 idx array [128, TOTAL/16], bf16 slot array [128, TTOT],
    needed T_BQ."""
    c = cfg
    b_e = ed // P
    q_e = es // c.CHUNK_REAL
    loc = (es % c.CHUNK_REAL).astype(np.int64)
    cnt = np.zeros((c.NB, c.NCH), np.int64)
    np.add.at(cnt, (b_e, q_e), 1)
    need = int(-(-cnt.max() // P))
    return b_e, q_e, loc, cnt, need


def _fill_streams(cfg, b_e, q_e, loc, slot_e, cnt):
    c = cfg
    # stream position of each (b, q) cell
    cell_off = np.zeros((c.NB, c.NCH), np.int64)
    off = 0
    for (b0, nblk) in c.GROUPS:
        for q in range(c.NCH):
            for br in range(nblk):
                cell_off[b0 + br, q] = off + br * c.CAP
            off += nblk * c.CAP
    assert off == c.TOTAL
    order = np.lexsort((q_e, b_e))
    bs, qs = b_e[order], q_e[order]
    cell_start = np.zeros((c.NB, c.NCH), np.int64)
    cell_start.reshape(-1)[1:] = np.cumsum(cnt.reshape(-1))[:-1]
    rank = np.arange(len(order)) - cell_start[bs, qs]
    pos = cell_off[bs, qs] + rank
    idx_flat = np.full(c.TOTAL, c.PAD_ROW, np.int16)
    slot_flat = np.zeros(c.TOTAL, np.float32)
    idx_flat[pos] = loc[order].astype(np.int16)
    slot_flat[pos] = slot_e[order]
    idxw = idx_flat.reshape(c.TOTAL // 16, 16).T.copy()          # [16, cols]
    idx_arr = np.tile(idxw, (8, 1))                              # [128, cols]
    slot_arr = (slot_flat.reshape(c.TTOT, P).T
                .astype(ml_dtypes.bfloat16).copy())              # [128, TTOT]
    return idx_arr, slot_arr


_NC_CACHE = {}


def _get_nc(key, builder):
    if key not in _NC_CACHE:
        _NC_CACHE[key] = builder()
    return _NC_CACHE[key]


def kernel(verts, edges, W0_1, W1_1, b1, W0_2, W1_2, b2):
    global LAST_EXEC_NS
    LAST_EXEC_NS = []
    N, IN_DIM = verts.shape
    E = edges.shape[0]
    NCORES = 8
    H1 = W0_1.shape[1]
    H2 = W0_2.shape[1]
    cfg = Cfg(N, E, NCORES, IN_DIM, H1, H2)

    verts = np.asarray(verts, np.float32)
    edges = np.asarray(edges)
    src = np.asarray(edges[:, 0], np.int64)
    dst = np.asarray(edges[:, 1], np.int64)
    bf = ml_dtypes.bfloat16

    deg = np.bincount(src, minlength=cfg.NPADCOL).astype(np.float32)
    deg_t = deg[:cfg.NCH * cfg.CHUNK_REAL].reshape(cfg.NT, P).T.copy()

    vt1 = np.zeros((IN_DIM, cfg.NPADCOL), bf)
    vt1[:, :N] = verts.T.astype(bf)

    iota = np.broadcast_to(np.arange(P, dtype=np.float32)[None, :],
                           (P, P)).astype(bf).copy()

    # per-core prep
    cores = []
    tbq_need = 1
    for ci in range(NCORES):
        lo = ci * cfg.OWN
        m = (dst >= lo) & (dst < lo + cfg.OWN)
        es, edl = src[m], dst[m] - lo
        b_e, q_e, loc, cnt, need = _schedule(cfg, es, edl)
        tbq_need = max(tbq_need, need)
        cores.append((lo, es, edl, b_e, q_e, loc, cnt))
    cfg.set_tbq(max(tbq_need, 1))

    in1_maps = []
    for (lo, es, edl, b_e, q_e, loc, cnt) in cores:
        idx_arr, slot_arr = _fill_streams(cfg, b_e, q_e, loc,
                                          (edl % P).astype(np.float32), cnt)
        degd = np.zeros((H1, cfg.NODES_PAD), np.float32)
        degd[:, :cfg.OWN] = deg[lo:lo + cfg.OWN][None, :]
        vox = np.zeros((IN_DIM, cfg.NODES_PAD), bf)
        hi = min(lo + cfg.NODES_PAD, N)
        vox[:, :hi - lo] = vt1[:, lo:hi]
        in1_maps.append({
            "vt": vt1, "w_t": W1_1.astype(bf), "w_d": W0_1.astype(bf),
            "bias": np.asarray(b1, np.float32).reshape(H1, 1),
            "deg_t": deg_t, "degd": degd, "idx": idx_arr, "slot": slot_arr,
            "iota": iota, "vox": vox,
        })

    trace = os.environ.get("MESHCONV_TRACE", "") == "1"

    nc1 = _get_nc(("l1", cfg.T_BQ),
                  lambda: _build_layer(cfg, IN_DIM, H1, True, False))
    r1 = run_bass_kernel_spmd(nc1, in1_maps, core_ids=list(range(NCORES)),
                              trace=trace)
    if trace and r1.exec_time_ns:
        LAST_EXEC_NS.append(r1.exec_time_ns)

    # assemble full h^T  [H1, NPADCOL] bf16
    ht = np.zeros((H1, cfg.NPADCOL), bf)
    for ci in range(NCORES):
        lo = ci * cfg.OWN
        ht[:, lo:lo + cfg.OWN] = r1.results[ci]["out"][:, :cfg.OWN]

    in2_maps = []
    for (lo, es, edl, b_e, q_e, loc, cnt), m1 in zip(cores, in1_maps):
        degd2 = np.zeros((H2, cfg.NODES_PAD), np.float32)
        degd2[:, :cfg.OWN] = deg[lo:lo + cfg.OWN][None, :]
        vox2 = np.zeros((H1, cfg.NODES_PAD), bf)
        hi = min(lo + cfg.NODES_PAD, cfg.NPADCOL)
        vox2[:, :hi - lo] = ht[:, lo:hi]
        in2_maps.append({
            "vt": ht, "w_t": W1_2.astype(bf), "w_d": W0_2.astype(bf),
            "bias": np.asarray(b2, np.float32).reshape(H2, 1),
            "deg_t": deg_t, "degd": degd2, "idx": m1["idx"],
            "slot": m1["slot"], "iota": iota,
            "vox": vox2,
        })

    nc2 = _get_nc(("l2", cfg.T_BQ),
                  lambda: _build_layer(cfg, H1, H2, False, True))
    r2 = run_bass_kernel_spmd(nc2, in2_maps, core_ids=list(range(NCORES)),
                              trace=trace)
    if trace and r2.exec_time_ns:
        LAST_EXEC_NS.append(r2.exec_time_ns)

    out = np.empty((N, H2), np.float32)
    for ci in range(NCORES):
        lo = ci * cfg.OWN
        out[lo:lo + cfg.OWN] = r2.results[ci]["out"][:, :cfg.OWN].T
    return out



# revision 7
# speedup vs baseline: 3.5434x; 3.5434x over previous
"""Trainium2 Bass kernel for nn_MeshConv (ChebConv K=2, two layers) on 8 cores.

Math (reference):
    deg  = bincount(src)
    dinv = where(deg>0, rsqrt(max(deg,1)), 0)
    Tx1  = segment_sum(-dinv[src]*dinv[dst] * x[src], dst)
    layer(x) = x @ W0 + Tx1 @ W1 + b     (layer1 wrapped in relu)

Device strategy (dst-sharded, two dispatches with a host hop):
  * no on-device gather-table build: the host ships node-major row tables
    (layer1: dinv[v]*x[v] rows padded to 512B; layer2: dinv[v]*h[v] rows
    padded to 256B, assembled from dispatch-1 output), split in 4 chunks
    of <=25216 rows for the int16 MoE dma_gather.
  * per 128-dst block, segment sum runs node-major on the PE:
        zx[128dst, kf] += s_oh[:,tt,:]^T @ g_rows[:,tt,0:kf]
    over the gathered 128-edge tiles (one-hot s_oh built on DVE).
  * epilogue per block: zsb = (-dinv[dst]) * zx  (ACT, per-partition scale),
    two PE transposes to get k on partitions, then a 4-matmul PSUM group
    acc = x_own @ W0 (+bias via ones-row) + zsb^T-chunks @ W1; h = relu(acc)
    on ACT; batched node-major output DMA per 6-block group.
"""
import numpy as np
import ml_dtypes

import concourse.bacc as bacc
import concourse.tile as tile
import concourse.mybir as mybir
from concourse import library_config
from concourse.bass_utils import run_bass_kernel_spmd

P = 128
ELEM1 = 256        # gather row width (bf16 elems) layer 1: 512B, 192 real
ELEM2 = 128        # gather row width layer 2: 256B, 64 real
SUB = 1024         # gather call granularity (SWDGE ring is 1024 descriptors)

LAST_EXEC_NS = []


class Cfg:
    def __init__(self, n_nodes, n_edges, n_cores, in_dim, h1, h2):
        self.N, self.E, self.C = n_nodes, n_edges, n_cores
        self.IN, self.H1, self.H2 = in_dim, h1, h2
        self.OWN = n_nodes // n_cores                 # owned dst nodes / core
        self.NB = -(-self.OWN // P)                   # dst blocks / core
        self.NODES_PAD = self.NB * P
        # src chunking for int16 gather indices
        self.NCH = -(-n_nodes // 25088) if n_nodes > 25088 else 1
        self.CHUNK_REAL = -(-n_nodes // self.NCH)
        self.CHUNK_REAL = -(-self.CHUNK_REAL // P) * P   # 128-aligned
        self.CHUNK_ROWS = self.CHUNK_REAL + P            # + zero pad tile
        assert self.CHUNK_ROWS <= 32767
        self.PAD_ROW = self.CHUNK_REAL                   # chunk-local zero row
        self.GRP = 6
        self.GROUPS = []
        b = 0
        while b < self.NB:
            n = min(self.GRP, self.NB - b)
            self.GROUPS.append((b, n))
            b += n

    def set_tbq(self, t_bq):
        self.T_BQ = t_bq
        self.CAP = t_bq * P
        self.TOTAL = self.NB * self.NCH * self.CAP       # idx slots / layer
        self.TTOT = self.TOTAL // P


def _build_layer(cfg, kf, m_out, relu, out_f32, elem):
    """One dispatch. kf: input feature dim (192 / 64); m_out: 64 / 32.
    elem: gather row width in bf16 elements (256 / 128)."""
    c = cfg
    nc = bacc.Bacc("TRN2", target_bir_lowering=False, debug=False)
    dt = mybir.dt

    # feature chunks of kf for transposes / projection matmuls
    kchunks = []
    r = 0
    while r < kf:
        n = min(P, kf - r)
        kchunks.append((r, n))
        r += n

    tables = [
        nc.dram_tensor(f"table{q}", [c.CHUNK_ROWS, elem], dt.bfloat16,
                       kind="ExternalInput")
        for q in range(c.NCH)
    ]
    # dense input: x_own^T chunks; last chunk has a trailing ones row
    vox = [
        nc.dram_tensor(f"vox{i}", [n + (1 if i == len(kchunks) - 1 else 0),
                                   c.NODES_PAD], dt.bfloat16,
                       kind="ExternalInput")
        for i, (r0, n) in enumerate(kchunks)
    ]
    # dense weights (last chunk augmented with bias row) and z weights
    wd = [
        nc.dram_tensor(f"wd{i}", [n + (1 if i == len(kchunks) - 1 else 0),
                                  m_out], dt.bfloat16, kind="ExternalInput")
        for i, (r0, n) in enumerate(kchunks)
    ]
    wz = [
        nc.dram_tensor(f"wz{i}", [n, m_out], dt.bfloat16, kind="ExternalInput")
        for i, (r0, n) in enumerate(kchunks)
    ]
    dinvneg = nc.dram_tensor("dinvneg", [P, c.NB], dt.float32,
                             kind="ExternalInput")
    dinvpos = nc.dram_tensor("dinvpos", [P, c.NB], dt.float32,
                             kind="ExternalInput")
    idx = nc.dram_tensor("idx", [P, c.TOTAL // 16], dt.int16,
                         kind="ExternalInput")
    slot = nc.dram_tensor("slot", [P, c.TTOT], dt.bfloat16,
                          kind="ExternalInput")
    iota = nc.dram_tensor("iota", [P, P], dt.bfloat16, kind="ExternalInput")
    ident = nc.dram_tensor("ident", [P, P], dt.bfloat16, kind="ExternalInput")
    odt = dt.float32 if out_f32 else dt.bfloat16
    out = nc.dram_tensor("out", [c.NB, P, m_out], odt, kind="ExternalOutput")

    AF = mybir.ActivationFunctionType
    with tile.TileContext(nc) as tc:
        with tc.tile_pool(name="const", bufs=1) as cpool:
            nc.gpsimd.load_library(library_config.mlp)

            iota_t = cpool.tile([P, 1, P], dt.bfloat16)
            nc.sync.dma_start(iota_t[:, 0, :], iota[:, :])
            ident_t = cpool.tile([P, P], dt.bfloat16)
            nc.sync.dma_start(ident_t[:], ident[:, :])
            dneg_t = cpool.tile([P, c.NB], dt.float32)
            nc.sync.dma_start(dneg_t[:], dinvneg[:, :])
            dpos_t = cpool.tile([P, c.NB], dt.float32)
            nc.sync.dma_start(dpos_t[:], dinvpos[:, :])
            wd_t = [cpool.tile([wd[i].shape[0], m_out], dt.bfloat16,
                               tag=f"wd{i}", name=f"wd{i}")
                    for i in range(len(kchunks))]
            wz_t = [cpool.tile([n, m_out], dt.bfloat16,
                               tag=f"wz{i}", name=f"wz{i}")
                    for i, (r0, n) in enumerate(kchunks)]
            for i in range(len(kchunks)):
                nc.sync.dma_start(wd_t[i][:], wd[i][:, :])
                nc.sync.dma_start(wz_t[i][:], wz[i][:, :])
            vox_t = [cpool.tile([vox[i].shape[0], c.NODES_PAD], dt.bfloat16,
                                tag=f"vox{i}", name=f"vox{i}")
                     for i in range(len(kchunks))]
            for i in range(len(kchunks)):
                nc.sync.dma_start(vox_t[i][:], vox[i][:, :])

            with tc.tile_pool(name="gat", bufs=2) as gpool, \
                 tc.tile_pool(name="epi", bufs=3) as epool, \
                 tc.tile_pool(name="zps", bufs=c.GRP, space="PSUM") as zpsum, \
                 tc.tile_pool(name="tps", bufs=1, space="PSUM") as tpsum, \
                 tc.tile_pool(name="ops", bufs=1, space="PSUM") as opsum:
                goff = 0       # idx entries consumed so far
                for (b0, nblk) in c.GROUPS:
                    nidx = nblk * c.CAP
                    tg = nidx // P
                    # one idx/slot load per group (covers all NCH chunks)
                    gn = c.NCH * nidx
                    idx_t = gpool.tile([P, gn // 16], dt.int16, tag="ix")
                    nc.sync.dma_start(
                        idx_t[:], idx[:, goff // 16:(goff + gn) // 16])
                    slot_t = gpool.tile([P, gn // P], dt.bfloat16, tag="sl")
                    nc.sync.dma_start(
                        slot_t[:], slot[:, goff // P:(goff + gn) // P])

                    zxs = [zpsum.tile([P, kf], dt.float32, space="PSUM",
                                      tag="zx", name="zx") for _ in range(nblk)]
                    for q in range(c.NCH):
                        g_t = gpool.tile([P, tg, elem], dt.bfloat16, tag="g")
                        i0 = q * nidx
                        for sb in range(0, nidx, SUB):
                            sn = min(SUB, nidx - sb)
                            nc.gpsimd.dma_gather(
                                g_t[:, sb // P:(sb + sn) // P, :],
                                tables[q][:, :],
                                idx_t[:, (i0 + sb) // 16:(i0 + sb + sn) // 16],
                                sn, sn, elem)
                        s_oh = gpool.tile([P, tg, P], dt.bfloat16, tag="s")
                        nc.vector.tensor_tensor(
                            out=s_oh[:],
                            in0=slot_t[:, i0 // P:(i0 + nidx) // P]
                                .to_broadcast([P, tg, P]),
                            in1=iota_t[:].to_broadcast([P, tg, P]),
                            op=mybir.AluOpType.is_equal)
                        for br in range(nblk):
                            for tr in range(c.T_BQ):
                                tt = br * c.T_BQ + tr
                                nc.tensor.matmul(
                                    out=zxs[br][:],
                                    lhsT=s_oh[:, tt, :],
                                    rhs=g_t[:, tt, 0:kf],
                                    start=(q == 0 and tr == 0),
                                    stop=(q == c.NCH - 1 and tr == c.T_BQ - 1))

                    ostage = epool.tile([P, nblk, m_out], odt, tag="ost")
                    for br in range(nblk):
                        blk = b0 + br
                        # zsb = (-dinv[dst]) * zx   [128, kf] bf16
                        zsb = epool.tile([P, kf], dt.bfloat16, tag="zsb")
                        nc.scalar.activation(
                            out=zsb[:], in_=zxs[br][:], func=AF.Copy,
                            scale=dneg_t[:, blk:blk + 1])
                        zts = []
                        for i, (r0, n) in enumerate(kchunks):
                            tp = tpsum.tile([P, P], dt.bfloat16, space="PSUM",
                                            tag="tp", name="tp")
                            nc.tensor.transpose(
                                tp[0:n, :], zsb[:, r0:r0 + n], ident_t[:])
                            zt = epool.tile([n, P], dt.bfloat16,
                                            tag=f"zt{i}", name=f"zt{i}")
                            nc.scalar.activation(out=zt[:], in_=tp[0:n, :],
                                                 func=AF.Copy)
                            zts.append(zt)
                        acc = opsum.tile([P, m_out], dt.float32, space="PSUM",
                                         tag="acc")
                        js = slice(blk * P, (blk + 1) * P)
                        for i in range(len(kchunks)):
                            nc.tensor.matmul(
                                out=acc[:], lhsT=vox_t[i][:, js],
                                rhs=wd_t[i][:], start=(i == 0), stop=False)
                        for i in range(len(kchunks)):
                            nc.tensor.matmul(
                                out=acc[:], lhsT=zts[i][:], rhs=wz_t[i][:],
                                start=False, stop=(i == len(kchunks) - 1))
                        nc.scalar.activation(
                            out=ostage[:, br, :], in_=acc[:],
                            func=AF.Relu if relu else AF.Copy)
                    nc.sync.dma_start(
                        out[b0:b0 + nblk, :, :].transpose([1, 0, 2]),
                        ostage[:])
                    goff += c.NCH * nidx
    nc.compile()
    return nc


def _schedule(cfg, es, ed):
    """Per-core edge schedule. es: src ids (global), ed: local dst ids."""
    c = cfg
    b_e = ed // P
    q_e = es // c.CHUNK_REAL
    loc = (es % c.CHUNK_REAL).astype(np.int64)
    cnt = np.zeros((c.NB, c.NCH), np.int64)
    np.add.at(cnt, (b_e, q_e), 1)
    need = int(-(-cnt.max() // P))
    return b_e, q_e, loc, cnt, need


def _fill_streams(cfg, b_e, q_e, loc, slot_e, cnt):
    c = cfg
    # stream position of each (b, q) cell; layout [group][q][block-in-group]
    cell_off = np.zeros((c.NB, c.NCH), np.int64)
    off = 0
    for (b0, nblk) in c.GROUPS:
        for q in range(c.NCH):
            for br in range(nblk):
                cell_off[b0 + br, q] = off + br * c.CAP
            off += nblk * c.CAP
    assert off == c.TOTAL
    order = np.lexsort((q_e, b_e))
    bs, qs = b_e[order], q_e[order]
    cell_start = np.zeros((c.NB, c.NCH), np.int64)
    cell_start.reshape(-1)[1:] = np.cumsum(cnt.reshape(-1))[:-1]
    rank = np.arange(len(order)) - cell_start[bs, qs]
    pos = cell_off[bs, qs] + rank
    idx_flat = np.full(c.TOTAL, c.PAD_ROW, np.int16)
    slot_flat = np.zeros(c.TOTAL, np.float32)
    idx_flat[pos] = loc[order].astype(np.int16)
    slot_flat[pos] = slot_e[order]
    idxw = idx_flat.reshape(c.TOTAL // 16, 16).T.copy()          # [16, cols]
    idx_arr = np.tile(idxw, (8, 1))                              # [128, cols]
    slot_arr = (slot_flat.reshape(c.TTOT, P).T
                .astype(ml_dtypes.bfloat16).copy())              # [128, TTOT]
    return idx_arr, slot_arr


_NC_CACHE = {}


def _get_nc(key, builder):
    if key not in _NC_CACHE:
        _NC_CACHE[key] = builder()
    return _NC_CACHE[key]


def _row_table(cfg, x_scaled, elem):
    """Node-major gather chunks [CHUNK_ROWS, elem] bf16 from scaled rows."""
    c = cfg
    kf = x_scaled.shape[1]
    bf = ml_dtypes.bfloat16
    chunks = []
    for q in range(c.NCH):
        t = np.zeros((c.CHUNK_ROWS, elem), bf)
        lo = q * c.CHUNK_REAL
        hi = min(lo + c.CHUNK_REAL, c.N)
        if hi > lo:
            t[:hi - lo, :kf] = x_scaled[lo:hi].astype(bf)
        chunks.append(t)
    return chunks


def kernel(verts, edges, W0_1, W1_1, b1, W0_2, W1_2, b2):
    global LAST_EXEC_NS
    LAST_EXEC_NS = []
    N, IN_DIM = verts.shape
    E = edges.shape[0]
    NCORES = 8
    H1 = W0_1.shape[1]
    H2 = W0_2.shape[1]
    cfg = Cfg(N, E, NCORES, IN_DIM, H1, H2)
    bf = ml_dtypes.bfloat16

    verts = np.asarray(verts, np.float32)
    src = np.asarray(edges[:, 0], np.int64)
    dst = np.asarray(edges[:, 1], np.int64)

    deg = np.bincount(src, minlength=N).astype(np.float64)
    dinv = np.where(deg > 0, 1.0 / np.sqrt(np.maximum(deg, 1.0)), 0.0)
    dinv = dinv.astype(np.float32)

    # per-core schedule
    cores = []
    tbq_need = 1
    for ci in range(NCORES):
        lo = ci * cfg.OWN
        m = (dst >= lo) & (dst < lo + cfg.OWN)
        es, edl = src[m], dst[m] - lo
        b_e, q_e, loc, cnt, need = _schedule(cfg, es, edl)
        tbq_need = max(tbq_need, need)
        cores.append((lo, b_e, q_e, loc, edl, cnt))
    cfg.set_tbq(max(tbq_need, 1))

    iota = np.broadcast_to(np.arange(P, dtype=np.float32)[None, :],
                           (P, P)).astype(bf).copy()
    ident = np.eye(P, dtype=np.float32).astype(bf)

    # ---- dispatch 1 host prep ----
    xs = verts * dinv[:, None]                        # dinv[src] * x rows
    tab1 = _row_table(cfg, xs, ELEM1)
    xT = verts.T.astype(bf)                           # [IN, N] for dense

    def kchunks_of(kf):
        ks, r = [], 0
        while r < kf:
            n = min(P, kf - r)
            ks.append((r, n))
            r += n
        return ks

    def vox_chunks(featT, kf, lo):
        # per-core dense input chunks; last chunk gets a ones row
        ks = kchunks_of(kf)
        outp = []
        hi = min(lo + cfg.NODES_PAD, featT.shape[1])
        for i, (r0, n) in enumerate(ks):
            last = i == len(ks) - 1
            v = np.zeros((n + (1 if last else 0), cfg.NODES_PAD), bf)
            v[:n, :hi - lo] = featT[r0:r0 + n, lo:hi]
            if last:
                own_n = min(cfg.OWN, cfg.NODES_PAD)
                v[n, :own_n] = 1.0
            outp.append(v)
        return outp

    def w_chunks(W, b, kf):
        ks = kchunks_of(kf)
        wds, wzs = [], []
        for i, (r0, n) in enumerate(ks):
            last = i == len(ks) - 1
            wdc = np.zeros((n + (1 if last else 0), W.shape[1]), bf)
            wdc[:n] = W[r0:r0 + n].astype(bf)
            if last:
                wdc[n] = np.asarray(b, np.float32).astype(bf)
            wds.append(wdc)
        return wds

    wd1 = w_chunks(W0_1, b1, IN_DIM)
    wz1 = [W1_1[r0:r0 + n].astype(bf) for (r0, n) in kchunks_of(IN_DIM)]
    wd2 = w_chunks(W0_2, b2, H1)
    wz2 = [W1_2[r0:r0 + n].astype(bf) for (r0, n) in kchunks_of(H1)]

    in1_maps, in2_static = [], []
    for (lo, b_e, q_e, loc, edl, cnt) in cores:
        idx_arr, slot_arr = _fill_streams(cfg, b_e, q_e, loc,
                                          (edl % P).astype(np.float32), cnt)
        dpv = np.zeros((P, cfg.NB), np.float32)
        own_n = min(cfg.OWN, cfg.NODES_PAD)
        dcol = np.zeros(cfg.NODES_PAD, np.float32)
        dcol[:own_n] = dinv[lo:lo + own_n]
        dpv[:, :] = dcol.reshape(cfg.NB, P).T
        m = {
            "dinvneg": -dpv, "dinvpos": dpv, "idx": idx_arr, "slot": slot_arr,
            "iota": iota, "ident": ident,
        }
        for q in range(cfg.NCH):
            m[f"table{q}"] = tab1[q]
        for i, v in enumerate(vox_chunks(xT, IN_DIM, lo)):
            m[f"vox{i}"] = v
        for i in range(len(wd1)):
            m[f"wd{i}"] = wd1[i]
            m[f"wz{i}"] = wz1[i]
        in1_maps.append(m)
        in2_static.append((lo, dict(dinvneg=-dpv, dinvpos=dpv, idx=idx_arr,
                                    slot=slot_arr, iota=iota, ident=ident)))

    import os
    trace = os.environ.get("MESHCONV_TRACE", "") == "1"

    nc1 = _get_nc(("l1", cfg.T_BQ),
                  lambda: _build_layer(cfg, IN_DIM, H1, True, False, ELEM1))
    r1 = run_bass_kernel_spmd(nc1, in1_maps, core_ids=list(range(NCORES)),
                              trace=trace)
    if trace and r1.exec_time_ns:
        LAST_EXEC_NS.append(r1.exec_time_ns)

    # ---- host hop: assemble h, build dispatch-2 tables ----
    h = np.zeros((N, H1), np.float32)
    for ci in range(NCORES):
        lo = ci * cfg.OWN
        hv = r1.results[ci]["out"].reshape(cfg.NODES_PAD, H1)
        h[lo:lo + cfg.OWN] = hv[:cfg.OWN].astype(np.float32)

    hs = h * dinv[:, None]
    tab2 = _row_table(cfg, hs, ELEM2)
    hT = h.T.astype(bf)

    in2_maps = []
    for (lo, stat) in in2_static:
        m = dict(stat)
        for q in range(cfg.NCH):
            m[f"table{q}"] = tab2[q]
        for i, v in enumerate(vox_chunks(hT, H1, lo)):
            m[f"vox{i}"] = v
        for i in range(len(wd2)):
            m[f"wd{i}"] = wd2[i]
            m[f"wz{i}"] = wz2[i]
        in2_maps.append(m)

    nc2 = _get_nc(("l2", cfg.T_BQ),
                  lambda: _build_layer(cfg, H1, H2, False, True, ELEM2))
    r2 = run_bass_kernel_spmd(nc2, in2_maps, core_ids=list(range(NCORES)),
                              trace=trace)
    if trace and r2.exec_time_ns:
        LAST_EXEC_NS.append(r2.exec_time_ns)

    out = np.empty((N, H2), np.float32)
    for ci in range(NCORES):
        lo = ci * cfg.OWN
        ov = r2.results[ci]["out"].reshape(cfg.NODES_PAD, H2)
        out[lo:lo + cfg.OWN] = ov[:cfg.OWN]
    return out
